# revision 2
# baseline (speedup 1.0000x reference)
"""TRN2 Bass kernel: nn_MultiHeadMusicDecoder (2-layer GRU decoder with
output feedback), data-parallel over batch on 8 NeuronCores.

Design: the feedback x_{t+1} = Wcat@h1_t + bcat is fused into layer-0 input
weights (A = Wih0@Wcat) so the T=512 sequential loop carries only (h0, h1);
the three heads are computed after the loop as one batched matmul. Gate rows
live on PSUM partitions, batch on the free dim; weights are the stationary
matmul operand (fp16, fast weight load, fp32 accumulate). All gate biases
are injected into PSUM by mask-matmuls that open each accumulation group.
Activation chain per layer: sig=sigmoid(P_rz); u=r*Q; v=u+P_ni (to PSUM);
n=tanh(v); zc=1-z; h' = z*h_prev + zc*n  (split per K-half so next-layer
matmuls start early).
"""


import numpy as np
from contextlib import ExitStack

import concourse.bass as bass
import concourse.bacc as bacc
import concourse.mybir as mybir
import concourse.tile as tile
import concourse.tile_rust as tile_rust

FP32 = mybir.dt.float32
AF = mybir.ActivationFunctionType

H = 256
NK = 2
NM = 6
WCOLS = NM * NK * 128


def const_layouts(B):
    W = 2 * B
    lay_h, lay_f = {}, {}
    off = 0
    for n_, w in [("W0i_t0", WCOLS), ("W0h", WCOLS), ("W1i", WCOLS),
                  ("W1h", WCOLS), ("W0i", WCOLS), ("Whead", 2 * H),
                  ("x0T", W),
                  ("bias4_0", 128), ("bias4_0t", 128), ("bias4_1", 128),
                  ("bias2i_0", 128), ("bias2i_0t", 128), ("bias2i_1", 128),
                  ("bias2h_0", 128), ("bias2h_1", 128),
                  ("mask4", 2 * W), ("mask2", W)]:
        lay_h[n_] = (off, w)
        off += w
    off = 0
    for n_, w in [("bcat_b", H)]:
        lay_f[n_] = (off, w)
        off += w
    return lay_h, lay_f


def arrange_lhsT(W):
    Wt = W.T
    cols = []
    for m in range(NM):
        for k in range(NK):
            cols.append(Wt[128 * k:128 * (k + 1), 128 * m:128 * (m + 1)])
    return np.ascontiguousarray(np.concatenate(cols, axis=1))


def lhsT_col(m, k):
    return (m * NK + k) * 128


def prep_core_inputs(inputs, dtype_np, b_slice):
    f32 = np.float32
    Wih0, Whh0 = np.asarray(inputs["Wih0"], f32), np.asarray(inputs["Whh0"], f32)
    Wih1, Whh1 = np.asarray(inputs["Wih1"], f32), np.asarray(inputs["Whh1"], f32)
    bih0, bhh0 = np.asarray(inputs["bih0"], f32), np.asarray(inputs["bhh0"], f32)
    bih1, bhh1 = np.asarray(inputs["bih1"], f32), np.asarray(inputs["bhh1"], f32)
    Wcat = np.concatenate([inputs["Wn"], inputs["Wd"], inputs["Wg"]], 0).astype(f32)
    bcat = np.concatenate([inputs["bn"], inputs["bd"], inputs["bg"]], 0).astype(f32)

    A = (Wih0.astype(np.float64) @ Wcat.astype(np.float64)).astype(f32)
    b_i0 = (Wih0.astype(np.float64) @ bcat.astype(np.float64)).astype(f32) + bih0

    x0 = np.asarray(inputs["initial_input"], f32)[b_slice, 0, :]
    B = x0.shape[0]
    W = 2 * B
    x0T = np.concatenate([x0.T[:128, :], x0.T[128:, :]], axis=1)

    def bias4(b_i, b_h):
        out = np.zeros((128, 128), f32)
        s = b_i[:2 * H] + b_h[:2 * H]
        for m in range(4):
            out[m, :] = s[m * 128:(m + 1) * 128]
        return out

    def bias2v(v):
        out = np.zeros((128, 128), f32)
        out[0, :] = v[:128]
        out[1, :] = v[128:]
        return out

    mask4 = np.zeros((128, 2 * W), f32)
    for m in range(4):
        mask4[m, m * B:(m + 1) * B] = 1.0
    mask2 = np.zeros((128, W), f32)
    for m in range(2):
        mask2[m, m * B:(m + 1) * B] = 1.0

    parts = {
        "W0i_t0": arrange_lhsT(Wih0),
        "W0i": arrange_lhsT(A),
        "W0h": arrange_lhsT(Whh0),
        "W1i": arrange_lhsT(Wih1),
        "W1h": arrange_lhsT(Whh1),
        "Whead": np.ascontiguousarray(
            np.concatenate([Wcat.T[:128, :], Wcat.T[128:, :]], 1)),
        "x0T": np.ascontiguousarray(x0T),
        "bias4_0": bias4(b_i0, bhh0),
        "bias4_0t": bias4(bih0, bhh0),
        "bias4_1": bias4(bih1, bhh1),
        "bias2i_0": bias2v(b_i0[2 * H:]),
        "bias2i_0t": bias2v(bih0[2 * H:]),
        "bias2i_1": bias2v(bih1[2 * H:]),
        "bias2h_0": bias2v(bhh0[2 * H:]),
        "bias2h_1": bias2v(bhh1[2 * H:]),
        "mask4": mask4,
        "mask2": mask2,
        "bcat_b": np.repeat(bcat[None, :], 128, 0),
    }
    lay_h, lay_f = const_layouts(B)
    CHm = np.concatenate([parts[n] for n in lay_h], axis=1).astype(dtype_np)
    CFm = np.concatenate([parts[n] for n in lay_f], axis=1).astype(f32)
    return {"CONSTH": np.ascontiguousarray(CHm), "CONSTF": np.ascontiguousarray(CFm)}


def build_nc(T=512, B=16, dtype=mybir.dt.float16, psum_bufs=3, act_bufs=3,
             reps=1, debug=False):
    nc = bacc.Bacc(None, debug=debug)
    dt2 = dtype
    W = 2 * B
    CT = 128 // B
    assert T % CT == 0

    lay_h, lay_f = const_layouts(B)
    wh = sum(w for _, w in lay_h.values())
    wf = sum(w for _, w in lay_f.values())
    CH = nc.declare_dram_parameter("CONSTH", [128, wh], dt2, isOutput=False)
    CF = nc.declare_dram_parameter("CONSTF", [128, wf], FP32, isOutput=False)
    Y_out = nc.declare_dram_parameter("Y", [B, T, H], FP32, isOutput=True)

    with tile.TileContext(nc) as tc, ExitStack() as ctx:
        cp = ctx.enter_context(tc.tile_pool(name="const", bufs=1))

        ch = cp.tile([128, wh], dt2, tag="CH", name="CH")
        cf = cp.tile([128, wf], FP32, tag="CF", name="CF")
        nc.sync.dma_start(ch[:], CH[:])
        nc.sync.dma_start(cf[:], CF[:])

        def sl(name, a, b, p0=0, p1=128):
            base, lay = (ch, lay_h) if name in lay_h else (cf, lay_f)
            off, w = lay[name]
            assert 0 <= a <= b <= w, (name, a, b, w)
            return base[p0:p1, off + a:off + b]

        h1hist = cp.tile([128, T * W], dt2, tag="h1hist", name="h1hist")
        h0 = [cp.tile([128, W], dt2, tag="h0a", name="h0a"),
              cp.tile([128, W], dt2, tag="h0b", name="h0b")]

        for rep in range(reps):
          with tc.tile_pool(name="psP", bufs=2, space="PSUM") as ppP, \
             tc.tile_pool(name="psN", bufs=2, space="PSUM") as ppN, \
             tc.tile_pool(name="psQ", bufs=2, space="PSUM") as ppQ, \
             tc.tile_pool(name="psV", bufs=2, space="PSUM") as ppV, \
             tc.tile_pool(name="act", bufs=act_bufs) as ap:

            for t in range(T):
                for layer in (0, 1):
                    if layer == 0:
                        Wi = "W0i_t0" if t == 0 else "W0i"
                        Wh = "W0h"
                        if t == 0:
                            rhs_i = lambda k: sl("x0T", k * B, (k + 1) * B)
                        else:
                            rhs_i = lambda k, _t=t: h1hist[
                                :, (_t - 1) * W + k * B:(_t - 1) * W + (k + 1) * B]
                        hp_tile = h0[(t - 1) % 2] if t > 0 else None
                        rhs_h = (lambda k, _h=hp_tile: _h[:, k * B:(k + 1) * B]) \
                            if t > 0 else None
                        h_prev = hp_tile[:] if t > 0 else None
                        h_dst_k = (lambda kk, _t=t: h0[_t % 2][:, kk * B:(kk + 1) * B])
                        b4 = "bias4_0t" if t == 0 else "bias4_0"
                        b2i = "bias2i_0t" if t == 0 else "bias2i_0"
                        b2h = "bias2h_0"
                    else:
                        Wi, Wh = "W1i", "W1h"
                        rhs_i = (lambda k, _t=t: h0[_t % 2][:, k * B:(k + 1) * B])
                        rhs_h = (lambda k, _t=t: h1hist[
                            :, (_t - 1) * W + k * B:(_t - 1) * W + (k + 1) * B]) \
                            if t > 0 else None
                        h_prev = h1hist[:, (t - 1) * W:t * W] if t > 0 else None
                        h_dst_k = (lambda kk, _t=t: h1hist[
                            :, _t * W + kk * B:_t * W + (kk + 1) * B])
                        b4, b2i, b2h = "bias4_1", "bias2i_1", "bias2h_1"

                    Prz = ppP.tile([128, 2 * W], FP32, tag="Prz", name="Prz")
                    Pni = ppN.tile([128, W], FP32, tag="Pni", name="Pni")
                    Q = ppQ.tile([128, W], FP32, tag="Q", name="Q")

                    # bias mask-matmuls open all three accumulation groups
                    nc.tensor.matmul(Prz[:], sl(b4, 0, 128, 0, 4),
                                     sl("mask4", 0, 2 * W, 0, 4),
                                     start=True, stop=False)
                    nc.tensor.matmul(Pni[:], sl(b2i, 0, 128, 0, 2),
                                     sl("mask2", 0, W, 0, 2),
                                     start=True, stop=False)
                    nc.tensor.matmul(Q[:], sl(b2h, 0, 128, 0, 2),
                                     sl("mask2", 0, W, 0, 2),
                                     start=True, stop=(rhs_h is None))

                    # h-side (skip at t==0)
                    if rhs_h is not None:
                        for m in range(4):
                            for k in range(NK):
                                nc.tensor.matmul(
                                    Prz[:, m * B:(m + 1) * B],
                                    sl(Wh, lhsT_col(m, k), lhsT_col(m, k) + 128),
                                    rhs_h(k), start=False, stop=False)
                        for m in range(2):
                            for k in range(NK):
                                nc.tensor.matmul(
                                    Q[:, m * B:(m + 1) * B],
                                    sl(Wh, lhsT_col(4 + m, k),
                                       lhsT_col(4 + m, k) + 128),
                                    rhs_h(k), start=False,
                                    stop=(m == 1 and k == NK - 1))
                    # i-side: rz first (sigmoid gate), ni afterwards
                    for m in range(4):
                        for k in range(NK):
                            nc.tensor.matmul(
                                Prz[:, m * B:(m + 1) * B],
                                sl(Wi, lhsT_col(m, k), lhsT_col(m, k) + 128),
                                rhs_i(k), start=False,
                                stop=(m == 3 and k == NK - 1))
                    for m in range(2):
                        for k in range(NK):
                            nc.tensor.matmul(
                                Pni[:, m * B:(m + 1) * B],
                                sl(Wi, lhsT_col(4 + m, k), lhsT_col(4 + m, k) + 128),
                                rhs_i(k), start=False,
                                stop=(m == 1 and k == NK - 1))

                    # ---- activation chain (biases already in PSUM) ----
                    sig = ap.tile([128, 2 * W], FP32, tag="sig", name="sig")
                    nc.scalar.activation(sig[:], Prz[:], AF.Sigmoid)
                    r = sig[:, 0:W]
                    z = sig[:, W:2 * W]
                    u = ap.tile([128, W], FP32, tag="u", name="u")
                    nc.vector.tensor_mul(u[:], r, Q[:])
                    v = ppV.tile([128, W], FP32, tag="v", name="v")
                    iv = nc.vector.tensor_add(v[:], u[:], Pni[:])
                    n_t = ap.tile([128, W], FP32, tag="n", name="n")
                    nc.scalar.activation(n_t[:], v[:], AF.Tanh)
                    zc = ap.tile([128, W], FP32, tag="zc", name="zc")
                    izc = nc.vector.tensor_scalar(zc[:], z, -1.0, 1.0,
                                                  mybir.AluOpType.mult,
                                                  mybir.AluOpType.add)
                    tile_rust.add_dep_helper(izc.ins, iv.ins, False,
                                             "keep v ahead of zc on DVE")

                    if h_prev is not None:
                        p1 = ap.tile([128, W], FP32, tag="p1", name="p1")
                        ip1 = nc.vector.tensor_mul(p1[:], z, h_prev)
                        tile_rust.add_dep_helper(ip1.ins, iv.ins, False,
                                                 "keep v ahead of p1 on DVE")
                        p2 = ap.tile([128, W], FP32, tag="p2", name="p2")
                        nc.vector.tensor_mul(p2[:], zc[:], n_t[:])
                        for kk in range(NK):
                            nc.vector.tensor_add(h_dst_k(kk),
                                                 p1[:, kk * B:(kk + 1) * B],
                                                 p2[:, kk * B:(kk + 1) * B])
                    else:
                        for kk in range(NK):
                            nc.vector.tensor_mul(h_dst_k(kk), zc[:, kk * B:(kk + 1) * B],
                                                 n_t[:, kk * B:(kk + 1) * B])

          # ---- head phase ----
          hist4 = h1hist[:].rearrange("p (t k b) -> p k t b", k=NK, b=B)
          h1k = [cp.tile([128, T * B], dt2, tag=f"h1k{k}", name=f"h1k{k}")
                 for k in range(NK)]
          RC = 8
          for c in range(RC):
              tt = slice(c * T // RC, (c + 1) * T // RC)
              for k in range(NK):
                  nc.vector.tensor_copy(
                      h1k[k][:, c * (T // RC) * B:(c + 1) * (T // RC) * B],
                      hist4[:, k, tt, :])
          with tc.tile_pool(name="hps", bufs=4, space="PSUM") as hp, \
               tc.tile_pool(name="hsb", bufs=4) as hb:
              for c in range(T // CT):
                  Yp = hp.tile([CT * B, H], FP32, tag="Yp", name="Yp")
                  for k in range(NK):
                      nc.tensor.matmul(Yp[:], h1k[k][:, c * 128:(c + 1) * 128],
                                       sl("Whead", k * H, (k + 1) * H),
                                       start=(k == 0), stop=(k == NK - 1))
                  Ysb = hb.tile([CT * B, H], FP32, tag="Ysb", name="Ysb")
                  nc.vector.tensor_add(Ysb[:], Yp[:], sl("bcat_b", 0, H))
                  nc.sync.dma_start(
                      Y_out[:, c * CT:(c + 1) * CT, :].rearrange("b t v -> t b v"),
                      Ysb[:])

    nc.compile()
    return nc


# ----------------------------------------------------------------------------
# Self-contained harness entry point: full inputs in, full outputs out.
# ----------------------------------------------------------------------------
from concourse.bass_utils import run_bass_kernel_spmd

T_FULL = 512
N_CORES = 8
B_CORE = 128 // N_CORES

_NC_CACHE = {}


def kernel(**inputs):
    """Full unsharded inputs (as reference.setup_inputs) -> (note, dur, gap)."""
    if "nc" not in _NC_CACHE:
        _NC_CACHE["nc"] = build_nc(T=T_FULL, B=B_CORE, dtype=mybir.dt.float16)
    nc = _NC_CACHE["nc"]

    in_maps = [prep_core_inputs(inputs, np.float16,
                                slice(c * B_CORE, (c + 1) * B_CORE))
               for c in range(N_CORES)]
    res = run_bass_kernel_spmd(nc, in_maps, list(range(N_CORES)))
    Y = np.concatenate([r["Y"] for r in res.results], axis=0)  # [128, T, 256]
    note = np.ascontiguousarray(Y[:, :, 0:128])
    dur = np.ascontiguousarray(Y[:, :, 128:192])
    gap = np.ascontiguousarray(Y[:, :, 192:256])
    return note, dur, gap


# revision 3
# speedup vs baseline: 1.0708x; 1.0708x over previous
"""TRN2 Bass kernel: nn_MultiHeadMusicDecoder (2-layer GRU decoder with
output feedback), data-parallel over batch on 8 NeuronCores.

Design: the feedback x_{t+1} = Wcat@h1_t + bcat is fused into layer-0 input
weights (A = Wih0@Wcat) so the T=512 sequential loop carries only (h0, h1);
the three heads are computed after the loop as one batched matmul. Gate rows
live on PSUM partitions, batch on the free dim; weights are the stationary
matmul operand (fp16, fast weight load, fp32 accumulate). All gate biases
are injected into PSUM by mask-matmuls that open each accumulation group.
Activation chain per layer: r=sigmoid(P_r) (r-region matmuls land first so
this starts early); u=r*Q; z=sigmoid(P_z) off-chain; v=u+P_ni (to PSUM);
n=tanh(v); zc=1-z; h' = z*h_prev + zc*n, split per K-half so the next
layer's matmuls start as soon as the first half lands.
"""


import numpy as np
from contextlib import ExitStack

import concourse.bass as bass
import concourse.bacc as bacc
import concourse.mybir as mybir
import concourse.tile as tile
import concourse.tile_rust as tile_rust

FP32 = mybir.dt.float32
AF = mybir.ActivationFunctionType

H = 256
NK = 2
NM = 6
WCOLS = NM * NK * 128


def const_layouts(B):
    W = 2 * B
    lay_h, lay_f = {}, {}
    off = 0
    for n_, w in [("W0i_t0", WCOLS), ("W0h", WCOLS), ("W1i", WCOLS),
                  ("W1h", WCOLS), ("W0i", WCOLS), ("Whead", 2 * H),
                  ("x0T", W),
                  ("bias4_0", 128), ("bias4_0t", 128), ("bias4_1", 128),
                  ("bias2i_0", 128), ("bias2i_0t", 128), ("bias2i_1", 128),
                  ("bias2h_0", 128), ("bias2h_1", 128),
                  ("mask4", 2 * W), ("mask2", W)]:
        lay_h[n_] = (off, w)
        off += w
    off = 0
    for n_, w in [("bcat_b", H)]:
        lay_f[n_] = (off, w)
        off += w
    return lay_h, lay_f


def arrange_lhsT(W):
    Wt = W.T
    cols = []
    for m in range(NM):
        for k in range(NK):
            cols.append(Wt[128 * k:128 * (k + 1), 128 * m:128 * (m + 1)])
    return np.ascontiguousarray(np.concatenate(cols, axis=1))


def lhsT_col(m, k):
    return (m * NK + k) * 128


def prep_core_inputs(inputs, dtype_np, b_slice):
    f32 = np.float32
    Wih0, Whh0 = np.asarray(inputs["Wih0"], f32), np.asarray(inputs["Whh0"], f32)
    Wih1, Whh1 = np.asarray(inputs["Wih1"], f32), np.asarray(inputs["Whh1"], f32)
    bih0, bhh0 = np.asarray(inputs["bih0"], f32), np.asarray(inputs["bhh0"], f32)
    bih1, bhh1 = np.asarray(inputs["bih1"], f32), np.asarray(inputs["bhh1"], f32)
    Wcat = np.concatenate([inputs["Wn"], inputs["Wd"], inputs["Wg"]], 0).astype(f32)
    bcat = np.concatenate([inputs["bn"], inputs["bd"], inputs["bg"]], 0).astype(f32)

    A = (Wih0.astype(np.float64) @ Wcat.astype(np.float64)).astype(f32)
    b_i0 = (Wih0.astype(np.float64) @ bcat.astype(np.float64)).astype(f32) + bih0

    x0 = np.asarray(inputs["initial_input"], f32)[b_slice, 0, :]
    B = x0.shape[0]
    W = 2 * B
    x0T = np.concatenate([x0.T[:128, :], x0.T[128:, :]], axis=1)

    def bias4(b_i, b_h):
        out = np.zeros((128, 128), f32)
        s = b_i[:2 * H] + b_h[:2 * H]
        for m in range(4):
            out[m, :] = s[m * 128:(m + 1) * 128]
        return out

    def bias2v(v):
        out = np.zeros((128, 128), f32)
        out[0, :] = v[:128]
        out[1, :] = v[128:]
        return out

    mask4 = np.zeros((128, 2 * W), f32)
    for m in range(4):
        mask4[m, m * B:(m + 1) * B] = 1.0
    mask2 = np.zeros((128, W), f32)
    for m in range(2):
        mask2[m, m * B:(m + 1) * B] = 1.0

    parts = {
        "W0i_t0": arrange_lhsT(Wih0),
        "W0i": arrange_lhsT(A),
        "W0h": arrange_lhsT(Whh0),
        "W1i": arrange_lhsT(Wih1),
        "W1h": arrange_lhsT(Whh1),
        "Whead": np.ascontiguousarray(
            np.concatenate([Wcat.T[:128, :], Wcat.T[128:, :]], 1)),
        "x0T": np.ascontiguousarray(x0T),
        "bias4_0": bias4(b_i0, bhh0),
        "bias4_0t": bias4(bih0, bhh0),
        "bias4_1": bias4(bih1, bhh1),
        "bias2i_0": bias2v(b_i0[2 * H:]),
        "bias2i_0t": bias2v(bih0[2 * H:]),
        "bias2i_1": bias2v(bih1[2 * H:]),
        "bias2h_0": bias2v(bhh0[2 * H:]),
        "bias2h_1": bias2v(bhh1[2 * H:]),
        "mask4": mask4,
        "mask2": mask2,
        "bcat_b": np.repeat(bcat[None, :], 128, 0),
    }
    lay_h, lay_f = const_layouts(B)
    CHm = np.concatenate([parts[n] for n in lay_h], axis=1).astype(dtype_np)
    CFm = np.concatenate([parts[n] for n in lay_f], axis=1).astype(f32)
    return {"CONSTH": np.ascontiguousarray(CHm), "CONSTF": np.ascontiguousarray(CFm)}


def build_nc(T=512, B=16, dtype=mybir.dt.float16, psum_bufs=3, act_bufs=3,
             reps=1, debug=False):
    nc = bacc.Bacc(None, debug=debug)
    dt2 = dtype
    W = 2 * B
    CT = 128 // B
    assert T % CT == 0

    lay_h, lay_f = const_layouts(B)
    wh = sum(w for _, w in lay_h.values())
    wf = sum(w for _, w in lay_f.values())
    CH = nc.declare_dram_parameter("CONSTH", [128, wh], dt2, isOutput=False)
    CF = nc.declare_dram_parameter("CONSTF", [128, wf], FP32, isOutput=False)
    Y_out = nc.declare_dram_parameter("Y", [B, T, H], FP32, isOutput=True)

    with tile.TileContext(nc) as tc, ExitStack() as ctx:
        cp = ctx.enter_context(tc.tile_pool(name="const", bufs=1))

        ch = cp.tile([128, wh], dt2, tag="CH", name="CH")
        cf = cp.tile([128, wf], FP32, tag="CF", name="CF")
        nc.sync.dma_start(ch[:], CH[:])
        nc.sync.dma_start(cf[:], CF[:])

        def sl(name, a, b, p0=0, p1=128):
            base, lay = (ch, lay_h) if name in lay_h else (cf, lay_f)
            off, w = lay[name]
            assert 0 <= a <= b <= w, (name, a, b, w)
            return base[p0:p1, off + a:off + b]

        h1hist = cp.tile([128, T * W], dt2, tag="h1hist", name="h1hist")
        h0 = [cp.tile([128, W], dt2, tag="h0a", name="h0a"),
              cp.tile([128, W], dt2, tag="h0b", name="h0b")]

        for rep in range(reps):
          with tc.tile_pool(name="psP", bufs=2, space="PSUM") as ppP, \
             tc.tile_pool(name="psN", bufs=2, space="PSUM") as ppN, \
             tc.tile_pool(name="psQ", bufs=2, space="PSUM") as ppQ, \
             tc.tile_pool(name="psV", bufs=2, space="PSUM") as ppV, \
             tc.tile_pool(name="act", bufs=act_bufs) as ap:

            for t in range(T):
                for layer in (0, 1):
                    if layer == 0:
                        Wi = "W0i_t0" if t == 0 else "W0i"
                        Wh = "W0h"
                        if t == 0:
                            rhs_i = lambda k: sl("x0T", k * B, (k + 1) * B)
                        else:
                            rhs_i = lambda k, _t=t: h1hist[
                                :, (_t - 1) * W + k * B:(_t - 1) * W + (k + 1) * B]
                        hp_tile = h0[(t - 1) % 2] if t > 0 else None
                        rhs_h = (lambda k, _h=hp_tile: _h[:, k * B:(k + 1) * B]) \
                            if t > 0 else None
                        h_prev = hp_tile[:] if t > 0 else None
                        h_dst_k = (lambda kk, _t=t: h0[_t % 2][:, kk * B:(kk + 1) * B])
                        b4 = "bias4_0t" if t == 0 else "bias4_0"
                        b2i = "bias2i_0t" if t == 0 else "bias2i_0"
                        b2h = "bias2h_0"
                    else:
                        Wi, Wh = "W1i", "W1h"
                        rhs_i = (lambda k, _t=t: h0[_t % 2][:, k * B:(k + 1) * B])
                        rhs_h = (lambda k, _t=t: h1hist[
                            :, (_t - 1) * W + k * B:(_t - 1) * W + (k + 1) * B]) \
                            if t > 0 else None
                        h_prev = h1hist[:, (t - 1) * W:t * W] if t > 0 else None
                        h_dst_k = (lambda kk, _t=t: h1hist[
                            :, _t * W + kk * B:_t * W + (kk + 1) * B])
                        b4, b2i, b2h = "bias4_1", "bias2i_1", "bias2h_1"

                    Prz = ppP.tile([128, 2 * W], FP32, tag="Prz", name="Prz")
                    Pni = ppN.tile([128, W], FP32, tag="Pni", name="Pni")
                    Q = ppQ.tile([128, W], FP32, tag="Q", name="Q")

                    # bias mask-matmuls open all three accumulation groups
                    nc.tensor.matmul(Prz[:], sl(b4, 0, 128, 0, 4),
                                     sl("mask4", 0, 2 * W, 0, 4),
                                     start=True, stop=False)
                    nc.tensor.matmul(Pni[:], sl(b2i, 0, 128, 0, 2),
                                     sl("mask2", 0, W, 0, 2),
                                     start=True, stop=False)
                    nc.tensor.matmul(Q[:], sl(b2h, 0, 128, 0, 2),
                                     sl("mask2", 0, W, 0, 2),
                                     start=True, stop=(rhs_h is None))

                    # h-side (skip at t==0)
                    if rhs_h is not None:
                        for m in range(4):
                            for k in range(NK):
                                nc.tensor.matmul(
                                    Prz[:, m * B:(m + 1) * B],
                                    sl(Wh, lhsT_col(m, k), lhsT_col(m, k) + 128),
                                    rhs_h(k), start=False, stop=False)
                        for m in range(2):
                            for k in range(NK):
                                nc.tensor.matmul(
                                    Q[:, m * B:(m + 1) * B],
                                    sl(Wh, lhsT_col(4 + m, k),
                                       lhsT_col(4 + m, k) + 128),
                                    rhs_h(k), start=False,
                                    stop=(m == 1 and k == NK - 1))
                    # i-side: rz first (sigmoid gate), ni afterwards
                    for m in range(4):
                        for k in range(NK):
                            nc.tensor.matmul(
                                Prz[:, m * B:(m + 1) * B],
                                sl(Wi, lhsT_col(m, k), lhsT_col(m, k) + 128),
                                rhs_i(k), start=False,
                                stop=(m == 3 and k == NK - 1))
                    for m in range(2):
                        for k in range(NK):
                            nc.tensor.matmul(
                                Pni[:, m * B:(m + 1) * B],
                                sl(Wi, lhsT_col(4 + m, k), lhsT_col(4 + m, k) + 128),
                                rhs_i(k), start=False,
                                stop=(m == 1 and k == NK - 1))

                    # ---- activation chain (biases already in PSUM) ----
                    sig = ap.tile([128, 2 * W], FP32, tag="sig", name="sig")
                    r = sig[:, 0:W]
                    z = sig[:, W:2 * W]
                    nc.scalar.activation(r, Prz[:, 0:W], AF.Sigmoid)
                    u = ap.tile([128, W], FP32, tag="u", name="u")
                    nc.vector.tensor_mul(u[:], r, Q[:])
                    nc.scalar.activation(z, Prz[:, W:2 * W], AF.Sigmoid)
                    v = ppV.tile([128, W], FP32, tag="v", name="v")
                    iv = nc.vector.tensor_add(v[:], u[:], Pni[:])
                    n_t = ap.tile([128, W], FP32, tag="n", name="n")
                    nc.scalar.activation(n_t[:], v[:], AF.Tanh)
                    zc = ap.tile([128, W], FP32, tag="zc", name="zc")
                    izc = nc.vector.tensor_scalar(zc[:], z, -1.0, 1.0,
                                                  mybir.AluOpType.mult,
                                                  mybir.AluOpType.add)
                    tile_rust.add_dep_helper(izc.ins, iv.ins, False,
                                             "keep v ahead of zc on DVE")

                    if h_prev is not None:
                        p1 = ap.tile([128, W], FP32, tag="p1", name="p1")
                        ip1 = nc.vector.tensor_mul(p1[:], z, h_prev)
                        tile_rust.add_dep_helper(ip1.ins, iv.ins, False,
                                                 "keep v ahead of p1 on DVE")
                        p2 = ap.tile([128, W], FP32, tag="p2", name="p2")
                        nc.vector.tensor_mul(p2[:], zc[:], n_t[:])
                        for kk in range(NK):
                            nc.vector.tensor_add(h_dst_k(kk),
                                                 p1[:, kk * B:(kk + 1) * B],
                                                 p2[:, kk * B:(kk + 1) * B])
                    else:
                        for kk in range(NK):
                            nc.vector.tensor_mul(h_dst_k(kk), zc[:, kk * B:(kk + 1) * B],
                                                 n_t[:, kk * B:(kk + 1) * B])

          # ---- head phase ----
          hist4 = h1hist[:].rearrange("p (t k b) -> p k t b", k=NK, b=B)
          h1k = [cp.tile([128, T * B], dt2, tag=f"h1k{k}", name=f"h1k{k}")
                 for k in range(NK)]
          RC = 8
          for c in range(RC):
              tt = slice(c * T // RC, (c + 1) * T // RC)
              for k in range(NK):
                  nc.vector.tensor_copy(
                      h1k[k][:, c * (T // RC) * B:(c + 1) * (T // RC) * B],
                      hist4[:, k, tt, :])
          with tc.tile_pool(name="hps", bufs=4, space="PSUM") as hp, \
               tc.tile_pool(name="hsb", bufs=4) as hb:
              for c in range(T // CT):
                  Yp = hp.tile([CT * B, H], FP32, tag="Yp", name="Yp")
                  for k in range(NK):
                      nc.tensor.matmul(Yp[:], h1k[k][:, c * 128:(c + 1) * 128],
                                       sl("Whead", k * H, (k + 1) * H),
                                       start=(k == 0), stop=(k == NK - 1))
                  Ysb = hb.tile([CT * B, H], FP32, tag="Ysb", name="Ysb")
                  nc.vector.tensor_add(Ysb[:], Yp[:], sl("bcat_b", 0, H))
                  nc.sync.dma_start(
                      Y_out[:, c * CT:(c + 1) * CT, :].rearrange("b t v -> t b v"),
                      Ysb[:])

    nc.compile()
    return nc


# ----------------------------------------------------------------------------
# Self-contained harness entry point: full inputs in, full outputs out.
# ----------------------------------------------------------------------------
from concourse.bass_utils import run_bass_kernel_spmd

T_FULL = 512
N_CORES = 8
B_CORE = 128 // N_CORES

_NC_CACHE = {}


def kernel(**inputs):
    """Full unsharded inputs (as reference.setup_inputs) -> (note, dur, gap)."""
    if "nc" not in _NC_CACHE:
        _NC_CACHE["nc"] = build_nc(T=T_FULL, B=B_CORE, dtype=mybir.dt.float16)
    nc = _NC_CACHE["nc"]

    in_maps = [prep_core_inputs(inputs, np.float16,
                                slice(c * B_CORE, (c + 1) * B_CORE))
               for c in range(N_CORES)]
    res = run_bass_kernel_spmd(nc, in_maps, list(range(N_CORES)))
    Y = np.concatenate([r["Y"] for r in res.results], axis=0)  # [128, T, 256]
    note = np.ascontiguousarray(Y[:, :, 0:128])
    dur = np.ascontiguousarray(Y[:, :, 128:192])
    gap = np.ascontiguousarray(Y[:, :, 192:256])
    return note, dur, gap
